# revision 33
# baseline (speedup 1.0000x reference)
"""Two-block single-head transformer (B=4, S=4096, E=256) on 8 TRN2 NeuronCores.

Sharding: core c -> batch b=c//2, query-half h=c%2 (2048 query rows each).
Each core receives its batch's x ROLLED so that its own query rows are always
rows [0:2048] -- this keeps the on-device program identical across cores
(pure SPMD, no partition-id branching).  Attention is permutation-invariant
over keys, so layer-1 may use the rolled key order.  Layer-2 keys: the own
half comes straight from this core's LN1 output (o_bounce); only the OTHER
half crosses the pairwise AllGather.

Math per layer (matches torch reference):
  q/k/v = x @ W.T + b ; att = softmax((q k^T)/sqrt(S)) ; o = att v
  layernorm over E with gamma/beta.

Performance structure (~2x over the bf16 baseline):
  * All matmuls run in fp8e4 with MatmulPerfMode.DoubleRow: the full E=256
    contraction in ONE PE instruction (2x bf16 FLOP rate; 259ns/512-col and
    133ns/257-col measured).  Layer-2 projection sources are DVE-cast
    bf16 -> fp8 after the transpose DMAs.
  * Scores for two consecutive key chunks accumulate into one 2-bank PSUM
    tile; a single wide [128,1024] EXP (Scalar engine) emits the fp8
    [128,2,512] tile that att@V uses directly as its DoubleRow lhsT.
    The Scalar engine is the pacing resource (16 EXPs ~ 20us per block).
  * K-projection bias dropped (softmax is invariant: (q+bq)@(k+bk)^T =
    (q+bq)@k^T + const(q)); V1 bias folded into the residual host-side
    (attention rows sum to 1).  PSUM->SBUF fp8 epilogues are split between
    Scalar (activation Identity, same act table as Exp) and DVE.
  * LN/residual path in bf16; y is bf16 on device, widened on the host.
  * Layer-boundary pipeline: per-block transposes are issued on the sync
    DMA queue as soon as their data lands; the dependent projections are
    mid-emitted a few attention pairs later so no in-order engine queue
    ever head-blocks on DMA.  A warm-up barrier collective absorbs the
    cross-core launch skew before the first real AllGather.
"""

import sys

sys.path.insert(0, "/opt/trn_rl_repo")

import numpy as np
import ml_dtypes

import concourse.bass as bass
import concourse.tile as tile
from concourse import bacc, mybir
from concourse import bass_utils

f32 = mybir.dt.float32
bf16 = mybir.dt.bfloat16
fp8 = mybir.dt.float8e4
DR = mybir.MatmulPerfMode.DoubleRow

B, S, E = 4, 4096, 256
P = 128
SQ = S // 2          # query rows per core
DC = E // P          # 2 chunks of the head dim
KC = S // P          # 32 key chunks
QB = 512             # query block (matmul moving dim)
NQB = SQ // QB       # 4 query blocks per core
QS = QB // P         # 4 query sub-blocks per block
NCH = S // QB        # 8 key chunks of 512
NPR = KC // 2        # 16 key-chunk pairs per attention block
N_CORES = 8
EPS = 1e-5
SCALE = 1.0 / np.sqrt(np.float32(S))

_COMPILED = None


def _broadcast_ap(vec_ap, parts, n):
    """[n] DRAM vector -> [parts, n] partition-broadcast access pattern."""
    return bass.AP(
        tensor=vec_ap.tensor,
        offset=vec_ap.offset,
        ap=[[0, parts], [1, n]],
    )


def _build():
    nc = bacc.Bacc(
        "TRN2", target_bir_lowering=False, debug=False, num_devices=N_CORES
    )

    # --- kernel I/O (per core) ---
    xT = nc.dram_tensor("xT", [P, DC * S], fp8, kind="ExternalInput").ap()
    xq = nc.dram_tensor("xq", [SQ, E], bf16, kind="ExternalInput").ap()
    wts = {
        n: nc.dram_tensor(n, [P, DC * E], fp8, kind="ExternalInput").ap()
        for n in ["wqt1", "wkt1", "wvt1", "wqt2", "wkt2", "wvt2"]
    }
    vecs = {
        n: nc.dram_tensor(n, [E], f32, kind="ExternalInput").ap()
        for n in ["bq1", "bq2", "bv2", "g1", "be1", "g2", "be2"]
    }
    ident = nc.dram_tensor("ident", [P, P], bf16, kind="ExternalInput").ap()
    y = nc.dram_tensor("y", [SQ, E], bf16, kind="ExternalOutput").ap()

    with tile.TileContext(nc) as tc:
        _emit(nc, tc, xT, xq, wts, vecs, ident, y)

    nc.compile()
    return nc


def _emit(nc, tc, xT, xq, wts, vecs, ident, y):
    from contextlib import ExitStack

    ctx = ExitStack()
    with ctx:
        const = ctx.enter_context(tc.tile_pool(name="const", bufs=1))
        srcT_pool = ctx.enter_context(tc.tile_pool(name="srcT", bufs=1))
        tr_pool = ctx.enter_context(tc.tile_pool(name="tr", bufs=1))
        kt_pool = ctx.enter_context(tc.tile_pool(name="kt", bufs=2))
        v_pool = ctx.enter_context(tc.tile_pool(name="v", bufs=2))
        qt_pool = ctx.enter_context(tc.tile_pool(name="qt", bufs=2))
        o_pool = ctx.enter_context(tc.tile_pool(name="okeep", bufs=16))
        work = ctx.enter_context(tc.tile_pool(name="work", bufs=4))
        expp = ctx.enter_context(tc.tile_pool(name="expp", bufs=3))
        stats = ctx.enter_context(tc.tile_pool(name="stats", bufs=8))
        # separate DRAM pools: dependency tracking is per-pool tensor, so a
        # collective writing one pool must not fence readers of another.
        dram_ob = [
            ctx.enter_context(tc.tile_pool(name=f"dram_ob{i}", bufs=1, space="DRAM"))
            for i in range(NQB)
        ]
        dram_ag = [
            ctx.enter_context(tc.tile_pool(name=f"dram_ag{i}", bufs=1, space="DRAM"))
            for i in range(NQB)
        ]
        dram_bar = ctx.enter_context(tc.tile_pool(name="dram_bar", bufs=1, space="DRAM"))
        # 2 double-width (2-bank) tiles: score pairs + projection outputs
        sc_ps = ctx.enter_context(tc.tile_pool(name="sc_ps", bufs=2, space="PSUM"))
        o_ps = ctx.enter_context(tc.tile_pool(name="o_ps", bufs=4, space="PSUM"))

        w_sb, bias_sb, bcast_sb = {}, {}, {}

        def _load_w(n):
            t = const.tile([P, DC, E], fp8, tag=f"w_{n}", name=f"w_{n}")
            nc.sync.dma_start(
                out=t[:], in_=wts[n].rearrange("p (dc o) -> p dc o", dc=DC)
            )
            w_sb[n] = t

        def _load_b(n):
            t = const.tile([P, DC], f32, tag=f"b_{n}", name=f"b_{n}")
            nc.sync.dma_start(
                out=t[:], in_=vecs[n].rearrange("(dc p) -> p dc", p=P)
            )
            bias_sb[n] = t

        def _load_bc(n, dt=f32):
            t = const.tile([P, E], dt, tag=f"bc_{n}", name=f"bc_{n}")
            nc.sync.dma_start(out=t[:], in_=_broadcast_ap(vecs[n], P, E))
            bcast_sb[n] = t

        # per-block LN1-output bounce buffers (own rows, rolled order)
        o_bounce = [
            dram_ob[i].tile([QB, E], bf16, name=f"obounce{i}")
            for i in range(NQB)
        ]
        # AllGather output per query block: rows [0:QB] = even-core rows
        # (canonical qb*QB..), rows [QB:2QB] = odd-core rows (SQ+qb*QB..).
        o_chunks = [
            dram_ag[i].tile([2 * QB, E], bf16, name=f"agchunk{i}")
            for i in range(NQB)
        ]

        # ---------------- projection helpers (all fp8 DoubleRow) --------
        def act_copy(out, in_, bias=0.0):
            return nc.scalar.activation(
                out=out, in_=in_, bias=bias,
                func=mybir.ActivationFunctionType.Identity,
            )

        def proj_k_chunk(kT_sb, srcT8, wk, dst_c, src_c, epi):
            """kT[:, :, dst_c*QB:+QB] from fp8 srcT8 cols [src_c*QB:+QB].
            No bias: softmax is invariant to the K bias."""
            ps = sc_ps.tile([P, 2, QB], f32, tag="mm", name="pk")
            for oc in range(DC):
                nc.tensor.matmul(
                    ps[:, oc, :],
                    lhsT=wk[:, :, oc * P:(oc + 1) * P],
                    rhs=srcT8[:, :, src_c * QB:(src_c + 1) * QB],
                    start=True, stop=True, perf_mode=DR,
                )
            dst = kT_sb[:, :, dst_c * QB:(dst_c + 1) * QB]
            if epi == "act":
                act_copy(out=dst, in_=ps[:])
            else:
                nc.vector.tensor_copy(out=dst, in_=ps[:])

        def proj_v_chunk(v_sb, srcT8, wv, dst_c, src_c, bv_bc=None):
            """V rows [dst_c*QB : +QB] (4 sub-chunks of 128)."""
            ps = sc_ps.tile([P, 4, E], f32, tag="mm", name="pv")
            for i in range(QS):
                sc = src_c * QS + i
                nc.tensor.matmul(
                    ps[:, i, :],
                    lhsT=srcT8[:, :, sc * P:(sc + 1) * P],
                    rhs=wv[:, :, :],
                    start=True, stop=True, perf_mode=DR,
                )
            if bv_bc is None:
                nc.vector.tensor_copy(
                    out=v_sb[:, dst_c * QS:(dst_c + 1) * QS, :E], in_=ps[:],
                )
            else:
                for i in range(QS):
                    nc.vector.tensor_add(
                        out=v_sb[:, dst_c * QS + i, :E],
                        in0=ps[:, i, :], in1=bv_bc[:],
                    )

        def proj_q_chunk(qT_out, srcT8, wq, bq, qc, epi):
            ps = sc_ps.tile([P, 2, QB], f32, tag="mm", name="pq")
            for oc in range(DC):
                nc.tensor.matmul(
                    ps[:, oc, :],
                    lhsT=wq[:, :, oc * P:(oc + 1) * P],
                    rhs=srcT8[:, :, qc * QB:(qc + 1) * QB],
                    start=True, stop=True, perf_mode=DR,
                )
            for oc in range(DC):
                dst = qT_out[:, oc, qc * QB:(qc + 1) * QB]
                if epi == "act":
                    act_copy(out=dst, in_=ps[:, oc, :], bias=bq[:, oc:oc + 1])
                else:
                    nc.vector.tensor_scalar_add(
                        out=dst, in0=ps[:, oc, :], scalar1=bq[:, oc:oc + 1],
                    )

        # ---------------- attention ----------------
        def attn_pair(kT_sb, v_sb, qT_sb, qb, po, pr, first, last):
            """Scores + exp + att@V for one consecutive key-chunk pair."""
            ps2 = sc_ps.tile([P, 2, QB], f32, tag="mm", name="sc")
            for j in range(2):
                kc = 2 * pr + j
                nc.tensor.matmul(
                    ps2[:, j, :],
                    lhsT=kT_sb[:, :, kc * P:(kc + 1) * P],
                    rhs=qT_sb[:, :, qb * QB:(qb + 1) * QB],
                    start=True, stop=True, perf_mode=DR,
                )
            ex = expp.tile([P, 2, QB], fp8, tag="exp")
            ins = nc.scalar.activation(
                out=ex.rearrange("p two f -> p (two f)")[:],
                in_=ps2.rearrange("p two f -> p (two f)")[:],
                func=mybir.ActivationFunctionType.Exp,
                scale=float(SCALE),
            )
            for qs in range(QS):
                nc.tensor.matmul(
                    po[qs][:],
                    lhsT=ex[:, :, qs * P:(qs + 1) * P],
                    rhs=v_sb[:, 2 * pr:2 * pr + 2, :],
                    start=first, stop=last, perf_mode=DR,
                )
            return ins

        def attn_epilogue(po, resid_tiles, g_bc, be_bc, out_cb, out_dst=None):
            """Residual + layernorm for one 512-query block (bf16 path).
            rstd via DVE-only Newton (3 iters from y0=1/var; var is within
            ~20% of 1 here) -- keeps Scalar on the Exp/Identity table."""
            resid = [f() for f in resid_tiles]
            ats, mvs = [], []
            var4 = stats.tile([P, QS], f32, tag="var4")
            mid_ins = None
            for qs in range(QS):
                den = stats.tile([P, 1], f32, tag="den")
                nc.vector.reciprocal(out=den[:], in_=po[qs][:, E:E + 1])
                at = work.tile([P, E], bf16, tag="attn", name=f"at{qs}")
                nc.vector.scalar_tensor_tensor(
                    out=at[:], in0=po[qs][:, :E], scalar=den[:], in1=resid[qs],
                    op0=mybir.AluOpType.mult, op1=mybir.AluOpType.add,
                )
                st = stats.tile([P, nc.vector.BN_STATS_DIM], f32, tag="bst")
                nc.vector.bn_stats(out=st[:], in_=at[:])
                mv = stats.tile([P, nc.vector.BN_AGGR_DIM], f32, tag="bag",
                                name=f"mv{qs}")
                nc.vector.bn_aggr(out=mv[:], in_=st[:])
                nc.vector.tensor_scalar_add(
                    out=var4[:, qs:qs + 1], in0=mv[:, 1:2], scalar1=EPS
                )
                ats.append(at)
                mvs.append(mv)
            rstd = stats.tile([P, QS], f32, tag="rstd")
            tmp = stats.tile([P, QS], f32, tag="nwt")
            nc.vector.reciprocal(out=rstd[:], in_=var4[:])
            for _ in range(3):
                nc.vector.tensor_mul(out=tmp[:], in0=rstd[:], in1=rstd[:])
                nc.vector.tensor_mul(out=tmp[:], in0=tmp[:], in1=var4[:])
                nc.vector.tensor_scalar(
                    out=tmp[:], in0=tmp[:], scalar1=-0.5, scalar2=1.5,
                    op0=mybir.AluOpType.mult, op1=mybir.AluOpType.add,
                )
                nc.vector.tensor_mul(out=rstd[:], in0=rstd[:], in1=tmp[:])
            for qs in range(QS):
                at = ats[qs]
                nc.vector.tensor_scalar(
                    out=at[:], in0=at[:],
                    scalar1=mvs[qs][:, 0:1], scalar2=rstd[:, qs:qs + 1],
                    op0=mybir.AluOpType.subtract, op1=mybir.AluOpType.mult,
                )
                nc.vector.tensor_mul(out=at[:], in0=at[:], in1=g_bc[:])
                dst = out_dst(qs) if out_dst else at
                ins = nc.vector.tensor_add(out=dst[:], in0=at[:], in1=be_bc[:])
                if qs == 1:
                    mid_ins = ins
                out_cb(qs, dst)
            return mid_ins

        # ---------------- tiles ----------------
        ident_box = []
        xT_sb = srcT_pool.tile([P, DC, S], fp8, tag="srcT")
        kT1 = kt_pool.tile([P, DC, S], fp8, tag="kt")
        v1 = v_pool.tile([P, KC, E + 1], fp8, tag="v")
        nc.vector.memset(v1[:, :, E:E + 1], 1.0)
        qT1 = qt_pool.tile([P, DC, SQ], fp8, tag="qt")
        xT_r = xT.rearrange("p (dc s) -> p dc s", dc=DC)

        # layer-2 sources.
        # oqT8: this core's own rows (Q2 source) -- transposed on the PE
        # (matmul is_transpose) because XBAR DMA transposes are serialized
        # behind ALL previously scheduled collectives by the framework;
        # oT: all 4096 rows canonical (K2/V2 source, from the AllGathers) --
        # their XBAR transposes genuinely depend on their own AllGather, so
        # each is emitted right after it and the forced collective wait is
        # exactly the true dependency.
        oqT8 = tr_pool.tile([P, DC, SQ], fp8, tag="oqT8")
        oT_bf = tr_pool.tile([P, DC, S], bf16, tag="oTbf")
        oT8 = srcT_pool.tile([P, DC, S], fp8, tag="oT8")
        kT2 = kt_pool.tile([P, DC, S], fp8, tag="kt")
        v2 = v_pool.tile([P, KC, E + 1], fp8, tag="v")
        nc.vector.memset(v2[:, :, E:E + 1], 1.0)
        qT2 = qt_pool.tile([P, DC, SQ], fp8, tag="qt")

        # absorb cross-core launch skew now, while only local work pends.
        bar_in = dram_bar.tile([1, 4], f32, name="bar_in")
        bar_out = dram_bar.tile([2, 4], f32, name="bar_out")
        nc.gpsimd.collective_compute(
            "AllGather",
            mybir.AluOpType.bypass,
            ins=[bar_in[:].opt()],
            outs=[bar_out[:].opt()],
            replica_groups=[[0, 1], [2, 3], [4, 5], [6, 7]],
        )

        po1 = [
            o_ps.tile([P, E + 1], f32, tag="ops", name=f"po1_{i}")
            for i in range(QS)
        ]
        po2 = [
            o_ps.tile([P, E + 1], f32, tag="ops", name=f"po2_{i}")
            for i in range(QS)
        ]
        resid1 = {}

        def load_resid1(qb, qs):
            t = work.tile([P, E], bf16, tag="xq")
            nc.sync.dma_start(
                out=t[:], in_=xq[(qb * QS + qs) * P:(qb * QS + qs + 1) * P, :]
            )
            return t

        o_tiles = []

        def dst1(qs):
            ot = o_pool.tile([P, E], bf16, tag="okeep")
            o_tiles.append(ot)
            return ot

        # ---------- layer-2 source pipeline pieces ----------
        def oq_pe_transpose(qb):
            """Transpose this block's own LN1 rows on the PE straight from
            the SBUF o_tiles; the PSUM->SBUF copy also casts to fp8."""
            pt = sc_ps.tile([P, DC, QS, P], bf16, tag="mm", name="ptr")
            for qs in range(QS):
                ot = o_tiles[qb * QS + qs]
                for dc in range(DC):
                    nc.tensor.matmul(
                        pt[:, dc, qs, :],
                        lhsT=ot[:, dc * P:(dc + 1) * P],
                        rhs=ident_box[0][:],
                        is_transpose=True,
                    )
            nc.vector.tensor_copy(
                out=oqT8[:, :, qb * QB:(qb + 1) * QB].rearrange(
                    "p dc (qs pp) -> p dc qs pp", qs=QS
                ),
                in_=pt[:],
            )

        def oT_transposes(qb):
            """oT columns for key chunks qb (even-core rows) and NQB+qb
            (odd-core rows), from AllGather qb -- canonical for all cores."""
            for half in range(2):
                r0 = half * SQ + qb * QB
                for dc in range(DC):
                    nc.sync.dma_start_transpose(
                        out=oT_bf[:, dc, r0:r0 + QB],
                        in_=o_chunks[qb][half * QB:(half + 1) * QB,
                                         dc * P:(dc + 1) * P],
                    )

        def q2_proj(qb):
            proj_q_chunk(qT2, oqT8, w_sb["wqt2"], bias_sb["bq2"], qb,
                         epi="dve")

        def cast_chunk(c, eng="pool"):
            # bf16 -> fp8 source cast, normally on the (otherwise idle) Pool
            # engine (~4.3us there but off the critical DVE/Scalar budgets);
            # the last AllGather's chunks use DVE (0.9us) to shorten the
            # post-collective chain.  Emitted well before the projections
            # that consume the fp8 tile.
            e = nc.gpsimd if eng == "pool" else nc.vector
            e.tensor_copy(
                out=oT8[:, :, c * QB:(c + 1) * QB],
                in_=oT_bf[:, :, c * QB:(c + 1) * QB],
            )

        def kv2p(c):
            proj_k_chunk(kT2, oT8, w_sb["wkt2"], c, c, epi="dve")
            proj_v_chunk(v2, oT8, w_sb["wvt2"], c, c, bv_bc=bcast_sb["bv2"])

        def kv2_proj(c, cast="pool"):
            cast_chunk(c, cast)
            kv2p(c)

        def all_gather(qb):
            nc.gpsimd.collective_compute(
                "AllGather",
                mybir.AluOpType.bypass,
                ins=[o_bounce[qb][:].opt()],
                outs=[o_chunks[qb][:].opt()],
                replica_groups=[[0, 1], [2, 3], [4, 5], [6, 7]],
            )

        # ---------------- the 8 attention blocks ----------------
        # Layer-2 consumes key-chunk pairs in AllGather arrival order:
        # chunks {0,4} both land with AG0, {1,5} with AG1, etc.
        pr_l2 = [c * 2 + j for c in [0, 4, 1, 5, 2, 6, 3, 7] for j in range(2)]
        pr_l1 = list(range(NPR))

        # mid[i] callbacks inject layer-2 source work into the pair loop so
        # in-order engine queues never head-block on DMA latency.
        def run_block(layer, qb, mid):
            kT_sb, v_sb, qT_sb, po, prs = (
                (kT1, v1, qT1, po1, pr_l1) if layer == 1
                else (kT2, v2, qT2, po2, pr_l2)
            )
            if layer == 1 and qb > 0:
                for qs in range(QS):
                    resid1[(qb, qs)] = load_resid1(qb, qs)
            if not (layer == 1 and qb == 0):  # block (1,0) pairs ran in P
                for i, pr in enumerate(prs):
                    if i in mid:
                        mid[i]()
                    attn_pair(kT_sb, v_sb, qT_sb, qb, po, pr,
                              first=(i == 0), last=(i == NPR - 1))
            if layer == 1:
                def out1(qs, ot):
                    r = qs * P
                    nc.sync.dma_start(out=o_bounce[qb][r:r + P, :], in_=ot[:])

                attn_epilogue(
                    po,
                    [lambda qb=qb, qs=qs: resid1[(qb, qs)]
                     for qs in range(QS)],
                    bcast_sb["g1"], bcast_sb["be1"], out1, out_dst=dst1,
                )
                all_gather(qb)
                oT_transposes(qb)
            else:
                def out2(qs, at):
                    r = (qb * QS + qs) * P
                    nc.sync.dma_start(out=y[r:r + P, :], in_=at[:])

                attn_epilogue(
                    po,
                    [lambda qs=qs, qb=qb: o_tiles[qb * QS + qs]
                     for qs in range(QS)],
                    bcast_sb["g2"], bcast_sb["be2"], out2,
                )

        # ------- phase P: layer-1 projections, interleaved with block 0 ----
        # chunk c's keys/values feed block-0 pairs 2c, 2c+1 immediately, so
        # the Scalar engine (the pacing resource) starts EXPs a few us in.
        _load_w("wqt1")
        nc.sync.dma_start(out=xT_sb[:, :, 0:QB], in_=xT_r[:, :, 0:QB])
        _load_w("wkt1")
        _load_w("wvt1")
        _load_b("bq1")
        for c in range(NCH):
            if c + 1 < NCH:
                nc.sync.dma_start(
                    out=xT_sb[:, :, (c + 1) * QB:(c + 2) * QB],
                    in_=xT_r[:, :, (c + 1) * QB:(c + 2) * QB],
                )
            if c == 2:
                for n in ["g1", "be1"]:
                    _load_bc(n)
                t = const.tile([P, P], bf16, tag="ident")
                nc.sync.dma_start(out=t[:], in_=ident)
                ident_box.append(t)
            if c == 4:
                for n in ["wqt2", "wkt2", "wvt2"]:
                    _load_w(n)
                _load_b("bq2")
                _load_bc("bv2")
                for n in ["g2", "be2"]:
                    _load_bc(n)
            if c < NQB:
                proj_q_chunk(qT1, xT_sb, w_sb["wqt1"], bias_sb["bq1"], c,
                             epi="act")
            proj_k_chunk(kT1, xT_sb, w_sb["wkt1"], c, c, epi="dve")
            proj_v_chunk(v1, xT_sb, w_sb["wvt1"], c, c)
            if c == 6:
                for qs in range(QS):
                    resid1[(0, qs)] = load_resid1(0, qs)
            for pr in (2 * c, 2 * c + 1):
                attn_pair(kT1, v1, qT1, 0, po1, pr,
                          first=(pr == 0), last=(pr == NPR - 1))

        run_block(1, 0, {})
        run_block(1, 1, {8: lambda: oq_pe_transpose(0),
                         12: lambda: q2_proj(0)})
        run_block(1, 2, {4: lambda: cast_chunk(0),
                         6: lambda: cast_chunk(4),
                         8: lambda: oq_pe_transpose(1),
                         12: lambda: q2_proj(1),
                         14: lambda: cast_chunk(1)})
        run_block(1, 3, {2: lambda: kv2p(0),
                         4: lambda: kv2p(4),
                         6: lambda: cast_chunk(5),
                         8: lambda: oq_pe_transpose(2),
                         10: lambda: kv2p(1),
                         12: lambda: q2_proj(2),
                         14: lambda: kv2p(5),
                         15: lambda: cast_chunk(2)})
        # L2 block 0 consumption order [0,4,1,5,2,6,3,7]: chunk 2 pairs at
        # i=8, chunk 6 at i=10, chunk 3 at i=12, chunk 7 at i=14.
        run_block(2, 0, {2: lambda: kv2p(2),
                         3: lambda: cast_chunk(6),
                         6: lambda: oq_pe_transpose(3),
                         8: lambda: kv2p(6),
                         10: lambda: q2_proj(3),
                         11: lambda: kv2_proj(3, cast="dve"),
                         13: lambda: kv2_proj(7, cast="dve")})
        for qb in range(1, NQB):
            run_block(2, qb, {})


def _prep_inputs(x, Wq1, bq1, Wk1, bk1, Wv1, bv1, Wq2, bq2, Wk2, bk2, Wv2,
                 bv2, g1, beta1, g2, beta2):
    bfl = ml_dtypes.bfloat16
    e4 = ml_dtypes.float8_e4m3
    shared = {}

    def _stripe(a2d, dt):
        e_in, n = a2d.shape
        return np.ascontiguousarray(
            a2d.reshape(DC, P, n).transpose(1, 0, 2).reshape(P, DC * n)
        ).astype(dt)

    for n, w in [("wqt1", Wq1), ("wkt1", Wk1), ("wvt1", Wv1),
                 ("wqt2", Wq2), ("wkt2", Wk2), ("wvt2", Wv2)]:
        shared[n] = _stripe(np.asarray(w, np.float32).T, e4)
    for n, v in [("bq1", bq1), ("bq2", bq2), ("bv2", bv2),
                 ("g1", g1), ("be1", beta1), ("g2", g2), ("be2", beta2)]:
        shared[n] = np.ascontiguousarray(np.asarray(v, np.float32))
    shared["ident"] = np.eye(P, dtype=bfl)

    x = np.asarray(x, np.float32)
    bv1 = np.asarray(bv1, np.float32)
    in_maps = []
    for c in range(N_CORES):
        b, h = c // 2, c % 2
        xb = x[b]
        if h:
            xb = np.concatenate([xb[SQ:], xb[:SQ]], axis=0)
        m = dict(shared)
        m["xT"] = _stripe(np.ascontiguousarray(xb.T), e4)
        # bv1 rides on the residual: softmax rows sum to 1, so
        # attn(v + bv) + x == attn(v) + (x + bv).
        m["xq"] = (xb[:SQ] + bv1).astype(bfl)
        in_maps.append(m)
    return in_maps


def _get_compiled():
    global _COMPILED
    if _COMPILED is None:
        _COMPILED = _build()
    return _COMPILED


def run(trace=False, **inputs):
    nc = _get_compiled()
    in_maps = _prep_inputs(**inputs)
    last_err = None
    for _ in range(3):
        try:
            res = bass_utils.run_bass_kernel_spmd(
                nc, in_maps, core_ids=list(range(N_CORES)), trace=trace
            )
            break
        except Exception as e:  # transient NRT device errors; retry
            last_err = e
    else:
        raise last_err
    out = np.empty((B, S, E), np.float32)
    for c in range(N_CORES):
        b, h = c // 2, c % 2
        out[b, h * SQ:(h + 1) * SQ] = np.asarray(
            res.results[c]["y"], dtype=np.float32
        )
    return out, res


def kernel(**inputs):
    out, _ = run(trace=False, **inputs)
    return out
